# revision 20
# baseline (speedup 1.0000x reference)
"""GRU-D Trainium2 kernel (8-core SPMD, data-parallel over batch).

Model (reference): B=512, T=200, D=128, H=512.
Per-core: 64 batch samples, full T recurrence.

v3: single fused pass, bf16 matmuls, latency-restructured scan.

Decomposition
-------------
h-independent terms are precomputed per 8-step chunk (phase A), kept in
SBUF (no DRAM round trip):
    delta_x = min(1, exp(-(d*w_gx + b_gx)))                  [elementwise]
    xhat    = m*x + (1-m)*(delta_x*xl + (1-delta_x)*xm)      [elementwise]
    dh      = min(1, exp(-(Wgh @ d + b_gh)))                 [D->H matmul]
    P_g     = Wgx_g @ xhat + Wgm_g @ m + b_g   for g in r,z,h

The serial scan consumes those records; per step (g = dh_t * h):
    r|z = sigmoid(P_{r,z} + W{r,z}h @ g)
    u  = r * g
    ht = tanh(P_h + Whh @ u)
    h' = g + z*(ht - g);   g_next = dh_{t+1} * h'
To shorten the post-tanh chain the scan tracks g (not h):
    a = dh' * g, b = dh' * z   (GpSimd, off critical path)
    g_next = a + b*(ht - g)    (3 half-width DVE ops, pipelined per
                                128-col pair so next step's matmuls
                                start on the first half of g_next)
P records are injected into PSUM with an identity matmul (start=True)
so sigmoid/tanh read PSUM directly (no DVE pre-add).

Phase A matmuls of chunk ci+1 are interleaved between scan steps of
chunk ci: they fill PE stall gaps and keep the HAM clock gate released
(PE stays at 2.4 GHz instead of the cold 1.2 GHz).

Everything on-device is feature-major: [H, B_local] tensors live as
SBUF tiles [128, 4*64] with column index = h_chunk*64 + b.

Final projection (H->2) + batch norm run on host over the gathered
h_last (trivial FLOPs, needs cross-core batch statistics anyway).
"""

import sys

for _p in ("/opt/trn_rl_repo",):
    if _p not in sys.path:
        sys.path.insert(0, _p)

import numpy as np
from ml_dtypes import bfloat16

import concourse.bacc as bacc
import concourse.tile as tile
from concourse import mybir

AF = mybir.ActivationFunctionType
F32 = mybir.dt.float32
BF16 = mybir.dt.bfloat16

B, T_FULL, D, H = 512, 200, 128, 512
NCORES = 8
BL = B // NCORES          # 64 samples per core
MC = H // 128             # 4 h-chunks
W = MC * BL               # 256 state columns
CHUNK = 512               # phase-A columns per chunk (= 8 steps)
TPC = CHUNK // BL         # timesteps per chunk (8)
BN_EPS = 1e-5

_nc_cache = {}


def build(T=T_FULL):
    assert T % TPC == 0
    TB = T * BL
    nchunk = TB // CHUNK

    nc = bacc.Bacc("TRN2", target_bir_lowering=False, debug=False)

    def din(name, shape, dt=F32):
        return nc.dram_tensor(name, shape, dt, kind="ExternalInput")

    x_d = din("x", [128, TB], BF16)
    xl_d = din("xl", [128, TB], BF16)
    m_d = din("m", [128, TB], BF16)
    dt_d = din("dt", [128, TB], BF16)
    xm_d = din("xm", [128, TB], BF16)

    wgx_d = din("wgx_n", [128, 1])      # -w_gx
    bgx_d = din("bgx_n", [128, 1])      # -b_gx
    wgh_d = din("wgh_t", [128, H], BF16)   # Wgh.T
    bgh_d = din("bgh_n", [128, MC])     # -b_gh  (col = h chunk)
    eye_d = din("eye", [128, 128], BF16)

    wxs_d = din("wx_t", [128, 3 * H], BF16)   # [Wrx.T | Wzx.T | Whx.T]
    wms_d = din("wm_t", [128, 3 * H], BF16)   # [Wrm.T | Wzm.T | Whm.T]
    whh_d = din("wh_t", [128, 3 * MC * H], BF16)  # r|z|h hidden blocks,
    #                                   tile (k,m) at g*4096 + k*512 + m*128
    bia_d = din("bias", [128, 3 * MC])  # b_r | b_z | b_h  (col = g*4 + chunk)

    h_out = nc.dram_tensor("h_out", [128, W], F32, kind="ExternalOutput")

    with tile.TileContext(nc) as tc:
        with (
            tc.tile_pool(name="wsb", bufs=1) as wp,
            tc.tile_pool(name="pin", bufs=3) as pin,
            tc.tile_pool(name="paw", bufs=2) as paw,
            tc.tile_pool(name="prec", bufs=2) as prp,
            tc.tile_pool(name="pg", bufs=2) as pg,
            tc.tile_pool(name="pb", bufs=2) as pb,
            tc.tile_pool(name="psA", bufs=2, space="PSUM") as psA,
            tc.tile_pool(name="psR", bufs=2, space="PSUM") as psR,
            tc.tile_pool(name="psZ", bufs=2, space="PSUM") as psZ,
            tc.tile_pool(name="psH", bufs=2, space="PSUM") as psH,
        ):
            # resident weights
            wgx = wp.tile([128, 1], F32, tag="wgx")
            bgx = wp.tile([128, 1], F32, tag="bgx")
            wgh = wp.tile([128, H], BF16, tag="wgh")
            bgh = wp.tile([128, MC], F32, tag="bgh")
            eye = wp.tile([128, 128], BF16, tag="eye")
            wxs = wp.tile([128, 3 * H], BF16, tag="wxs")
            wms = wp.tile([128, 3 * H], BF16, tag="wms")
            whh = wp.tile([128, 3 * MC * H], BF16, tag="whh")
            bia = wp.tile([128, 3 * MC], F32, tag="bia")
            for sb_t, dr in [
                (wgx, wgx_d), (bgx, bgx_d), (wgh, wgh_d), (bgh, bgh_d),
                (eye, eye_d), (wxs, wxs_d), (wms, wms_d), (whh, whh_d),
                (bia, bia_d),
            ]:
                nc.sync.dma_start(sb_t[:], dr[:])

            # ---- phase A emission helpers -------------------------------
            # Phase A for chunk ci is emitted piecewise: ew ops (DVE/Act)
            # up front, matmul+drain closures doled out between scan steps.
            def phaseA_start(ci):
                """DMA + elementwise for chunk ci; returns state dict."""
                s = ci * CHUNK
                xt = pin.tile([128, CHUNK], BF16, tag="x")
                xlt = pin.tile([128, CHUNK], BF16, tag="xl")
                mt = pin.tile([128, CHUNK], BF16, tag="m")
                dtt = pin.tile([128, CHUNK], BF16, tag="d")
                xmt = pin.tile([128, CHUNK], BF16, tag="xm")
                nc.sync.dma_start(xt[:], x_d[:, s:s + CHUNK])
                nc.sync.dma_start(xlt[:], xl_d[:, s:s + CHUNK])
                nc.sync.dma_start(mt[:], m_d[:, s:s + CHUNK])
                nc.sync.dma_start(dtt[:], dt_d[:, s:s + CHUNK])
                nc.sync.dma_start(xmt[:], xm_d[:, s:s + CHUNK])
                dh_sb = prp.tile([128, TPC, W], BF16, tag="dh")
                prec = prp.tile([128, TPC, 3 * W], BF16, tag="prec")
                st = dict(xt=xt, xlt=xlt, mt=mt, dtt=dtt, xmt=xmt,
                          dh_sb=dh_sb, prec=prec)
                return st

            def phaseA_ew(st):
                """Elementwise xhat chain (list of closures)."""
                xt, xlt, mt, dtt, xmt = (st["xt"], st["xlt"], st["mt"],
                                         st["dtt"], st["xmt"])
                dxe = paw.tile([128, CHUNK], F32, tag="dxe")
                dx = paw.tile([128, CHUNK], F32, tag="dx")
                t1 = paw.tile([128, CHUNK], F32, tag="t1")
                t2 = paw.tile([128, CHUNK], F32, tag="t2")
                t2b = paw.tile([128, CHUNK], F32, tag="t2b")
                t3 = paw.tile([128, CHUNK], F32, tag="t3")
                t4 = paw.tile([128, CHUNK], F32, tag="t4")
                xh = paw.tile([128, CHUNK], BF16, tag="xh")
                st["xh"] = xh
                return [
                    lambda: nc.scalar.activation(
                        dxe[:], dtt[:], AF.Exp, bias=bgx[:, 0:1],
                        scale=wgx[:, 0:1]),
                    lambda: nc.gpsimd.tensor_scalar_min(dx[:], dxe[:], 1.0),
                    lambda: nc.gpsimd.tensor_sub(t1[:], xlt[:], xmt[:]),
                    lambda: nc.gpsimd.tensor_mul(t2[:], dx[:], t1[:]),
                    lambda: nc.gpsimd.tensor_add(t2b[:], t2[:], xmt[:]),
                    lambda: nc.vector.tensor_sub(t3[:], xt[:], t2b[:]),
                    lambda: nc.vector.tensor_mul(t4[:], mt[:], t3[:]),
                    lambda: nc.vector.tensor_add(xh[:], t4[:], t2b[:]),
                ]

            def phaseA_mms(st):
                """Matmul + drain closures for chunk ci (run between steps).

                Order: 4x (dh matmul + exp drain), then DVE min, then
                12x (P_g pair matmul + bias drain)."""
                dtt, mt, dh_sb, prec = (st["dtt"], st["mt"], st["dh_sb"],
                                        st["prec"])
                ops = []

                def dh_mm(mi):
                    def f():
                        pdm = psA.tile([128, CHUNK], F32, tag="psA")
                        nc.tensor.matmul(
                            pdm[:], wgh[:, mi * 128:(mi + 1) * 128], dtt[:],
                            start=True, stop=True)
                        nc.scalar.activation(
                            dh_sb[:, :, mi * BL:(mi + 1) * BL],
                            pdm[:].rearrange("p (t b) -> p t b", b=BL),
                            AF.Exp, bias=bgh[:, mi:mi + 1], scale=-1.0)
                    return f

                for mi in range(MC):
                    ops.append(dh_mm(mi))

                def dh_min(tt):
                    def f():
                        sl = st["dh_sb"][:, 2 * tt:2 * tt + 2, :]
                        nc.gpsimd.tensor_scalar_min(sl, sl, 1.0)
                    return f

                for tt in range(TPC // 2):
                    ops.append(dh_min(tt))

                def pg_mm(gi, mi):
                    def f():
                        xh = st["xh"]
                        pp = psA.tile([128, CHUNK], F32, tag="psA")
                        wcol = gi * H + mi * 128
                        nc.tensor.matmul(
                            pp[:], wms[:, wcol:wcol + 128], mt[:],
                            start=True, stop=False)
                        nc.tensor.matmul(
                            pp[:], wxs[:, wcol:wcol + 128], xh[:],
                            start=False, stop=True)
                        dst = prec[:, :, gi * W + mi * BL:
                                   gi * W + (mi + 1) * BL]
                        src = pp[:].rearrange("p (t b) -> p t b", b=BL)
                        b_ap = bia[:, gi * MC + mi:gi * MC + mi + 1]
                        if (gi * MC + mi) % 2 == 0:
                            nc.scalar.activation(dst, src, AF.Identity,
                                                 bias=b_ap)
                        else:
                            nc.vector.tensor_scalar_add(dst, src, b_ap)
                    return f

                for gi in range(3):
                    for mi in range(MC):
                        ops.append(pg_mm(gi, mi))
                return ops

            # ---- the fused loop ----------------------------------------
            # rotating scan state: g tile (bf16) per step
            g_cur = pg.tile([128, W], BF16, tag="g0")
            nc.vector.memset(g_cur[:], 0.0)

            stA = phaseA_start(0)
            for f in phaseA_ew(stA):
                f()
            for f in phaseA_mms(stA):
                f()

            GATE_R, GATE_Z, GATE_H = 0, 1, 2

            def step(t, st_cur, st_next, apool):
                """One scan step; st_cur holds records for t, st_next for
                t+1 (same chunk or next). apool: phase-A closures to
                interleave. Returns new g tile."""
                nonlocal g_cur
                s = t % TPC
                prec = st_cur["prec"]
                dh_sb = st_cur["dh_sb"]
                last = t == T - 1
                if not last:
                    s_n = (t + 1) % TPC
                    dh_next = st_next["dh_sb"][:, s_n, :]

                def drain(n):
                    for _ in range(n):
                        if apool:
                            apool.pop(0)()

                # --- r gate ---
                pr = psR.tile([128, W], F32, tag="pr")
                nc.tensor.matmul(pr[:], eye[:], prec[:, s, 0:W],
                                 start=True, stop=False)
                for k in range(MC):
                    gk = g_cur[:, k * BL:(k + 1) * BL]
                    for mi in range(MC):
                        wcol = GATE_R * MC * H + k * H + mi * 128
                        nc.tensor.matmul(
                            pr[:, mi * BL:(mi + 1) * BL],
                            whh[:, wcol:wcol + 128], gk,
                            start=False,
                            stop=(k == MC - 1 and mi == MC - 1))
                r_sb = pb.tile([128, W], F32, tag="r")
                nc.scalar.activation(r_sb[:], pr[:], AF.Sigmoid)
                u = pb.tile([128, W], BF16, tag="u")
                nc.vector.tensor_mul(u[:], r_sb[:], g_cur[:])
                if not last:
                    a_t = pb.tile([128, W], F32, tag="a")
                    nc.gpsimd.tensor_mul(a_t[:], dh_next, g_cur[:])

                # --- z gate (PE busy while sigmoid/u run) ---
                pz = psZ.tile([128, W], F32, tag="pz")
                nc.tensor.matmul(pz[:], eye[:], prec[:, s, W:2 * W],
                                 start=True, stop=False)
                for k in range(MC):
                    gk = g_cur[:, k * BL:(k + 1) * BL]
                    for mi in range(MC):
                        wcol = GATE_Z * MC * H + k * H + mi * 128
                        nc.tensor.matmul(
                            pz[:, mi * BL:(mi + 1) * BL],
                            whh[:, wcol:wcol + 128], gk,
                            start=False,
                            stop=(k == MC - 1 and mi == MC - 1))
                drain(2)
                z_sb = pb.tile([128, W], F32, tag="z")
                nc.scalar.activation(z_sb[:], pz[:], AF.Sigmoid)
                if not last:
                    b_t = pb.tile([128, W], F32, tag="b")
                    nc.gpsimd.tensor_mul(b_t[:], dh_next, z_sb[:])

                # --- candidate ---
                ph = psH.tile([128, W], F32, tag="ph")
                nc.tensor.matmul(ph[:], eye[:], prec[:, s, 2 * W:3 * W],
                                 start=True, stop=False)
                for mi in range(MC):
                    for k in range(MC):
                        wcol = GATE_H * MC * H + k * H + mi * 128
                        nc.tensor.matmul(
                            ph[:, mi * BL:(mi + 1) * BL],
                            whh[:, wcol:wcol + 128],
                            u[:, k * BL:(k + 1) * BL],
                            start=False,
                            stop=(mi == MC - 1 and k == MC - 1))
                drain(2)

                # --- tanh + combine, per 128-col pair ---
                g_new = pg.tile([128, W], BF16, tag=f"g{(t + 1) % 2}")
                hts = pb.tile([128, W], F32, tag="hts")
                d1 = pb.tile([128, W], F32, tag="d1")
                if last:
                    d2 = pb.tile([128, W], F32, tag="d2")
                    hfin = pb.tile([128, W], F32, tag="hfin")
                for half in range(2):
                    c0, c1 = half * 128, (half + 1) * 128
                    nc.scalar.activation(hts[:, c0:c1], ph[:, c0:c1],
                                         AF.Tanh)
                    nc.vector.tensor_sub(d1[:, c0:c1], hts[:, c0:c1],
                                         g_cur[:, c0:c1])
                    if last:
                        nc.vector.tensor_mul(d2[:, c0:c1], z_sb[:, c0:c1],
                                             d1[:, c0:c1])
                        nc.vector.tensor_add(hfin[:, c0:c1], g_cur[:, c0:c1],
                                             d2[:, c0:c1])
                    else:
                        nc.vector.tensor_mul(d1[:, c0:c1], b_t[:, c0:c1],
                                             d1[:, c0:c1])
                        nc.vector.tensor_add(g_new[:, c0:c1], a_t[:, c0:c1],
                                             d1[:, c0:c1])
                if last:
                    nc.sync.dma_start(h_out[:], hfin[:])
                g_cur = g_new

            stB = stA
            apool = []
            for ci in range(nchunk):
                st_next = None
                if ci + 1 < nchunk:
                    st_next = phaseA_start(ci + 1)
                    apool = phaseA_ew(st_next) + phaseA_mms(st_next)
                else:
                    apool = []
                for s in range(TPC):
                    t = ci * TPC + s
                    nxt = st_next if s == TPC - 1 else stB
                    step(t, stB, nxt, apool)
                for f in apool:  # any leftovers
                    f()
                apool = []
                stB = st_next

    nc.compile()
    return nc


def get_nc(T=T_FULL):
    if T not in _nc_cache:
        _nc_cache[T] = build(T)
    return _nc_cache[T]


# ---------------------------------------------------------------- host prep

def _feature_major(a, Tn):
    """[BL, T, D] -> [D, T*BL] with b fastest."""
    return np.ascontiguousarray(
        a.transpose(2, 1, 0), bfloat16).reshape(D, Tn * BL)


def prep_shared(W_gh, b_gh, W_z, b_z, W_r, b_r, W_h, b_h, w_gx, b_gx):
    """Weight arrays shared by all cores (host layout). Gate order r,z,h."""
    def split(Wf):
        return Wf[:, :D], Wf[:, D:D + H], Wf[:, D + H:]

    Wzx, Wzh, Wzm = split(W_z)
    Wrx, Wrh, Wrm = split(W_r)
    Whx, Whh_, Whm = split(W_h)

    def hid_t(Wh):
        # Wh [H, H] -> Wh.T tiles: [128, MC*H] with tile (k,m) at k*H + m*128
        return (
            Wh.T.reshape(MC, 128, H).transpose(1, 0, 2).reshape(128, MC * H)
        )

    f32 = np.float32
    return {
        "wgx_n": np.ascontiguousarray(-w_gx[:, None], f32),
        "bgx_n": np.ascontiguousarray(-b_gx[:, None], f32),
        "wgh_t": np.ascontiguousarray(W_gh.T, bfloat16),
        "bgh_n": np.ascontiguousarray(-b_gh.reshape(MC, 128).T, f32),
        "eye": np.ascontiguousarray(np.eye(128), bfloat16),
        "wx_t": np.ascontiguousarray(
            np.concatenate([Wrx.T, Wzx.T, Whx.T], axis=1), bfloat16),
        "wm_t": np.ascontiguousarray(
            np.concatenate([Wrm.T, Wzm.T, Whm.T], axis=1), bfloat16),
        "wh_t": np.ascontiguousarray(
            np.concatenate([hid_t(Wrh), hid_t(Wzh), hid_t(Whh_)], axis=1),
            bfloat16),
        "bias": np.ascontiguousarray(
            np.concatenate(
                [b.reshape(MC, 128).T for b in (b_r, b_z, b_h)], axis=1), f32),
    }


def prep_core(X, X_last_obsv, Mask, Delta, xm_fm, shared, c, Tn):
    sl = slice(c * BL, (c + 1) * BL)
    m = {
        "x": _feature_major(X[sl], Tn),
        "xl": _feature_major(X_last_obsv[sl], Tn),
        "m": _feature_major(Mask[sl], Tn),
        "dt": _feature_major(Delta[sl], Tn),
        "xm": xm_fm,
    }
    m.update(shared)
    return m


def host_finish(h_outs, W_fc, b_fc, bn_gamma, bn_beta):
    """Gather per-core h_last, project to logits, batch-norm over batch."""
    h_last = np.concatenate(
        [o.reshape(128, MC, BL).transpose(2, 1, 0).reshape(BL, H)
         for o in h_outs], axis=0)                      # [B, H]
    logits = h_last.astype(np.float32) @ W_fc.T.astype(np.float32) + b_fc
    mu = logits.mean(axis=0)
    var = logits.var(axis=0)
    out = bn_gamma * (logits - mu) / np.sqrt(var + BN_EPS) + bn_beta
    return out.astype(np.float32)


def run_cores(inputs, Tn=T_FULL, trace=False):
    from concourse.bass_utils import run_bass_kernel_spmd

    inputs = {k: np.asarray(v, dtype=np.float32) for k, v in inputs.items()}
    nc = get_nc(Tn)
    shared = prep_shared(
        inputs["W_gh"], inputs["b_gh"], inputs["W_z"], inputs["b_z"],
        inputs["W_r"], inputs["b_r"], inputs["W_h"], inputs["b_h"],
        inputs["w_gx"], inputs["b_gx"],
    )
    xm_fm = np.ascontiguousarray(
        np.broadcast_to(
            inputs["x_mean"].transpose(2, 1, 0), (D, Tn, BL)
        ), bfloat16).reshape(D, Tn * BL)
    in_maps = [
        prep_core(inputs["X"], inputs["X_last_obsv"], inputs["Mask"],
                  inputs["Delta"], xm_fm, shared, c, Tn)
        for c in range(NCORES)
    ]
    res = run_bass_kernel_spmd(
        nc, in_maps, list(range(NCORES)), trace=trace,
    )
    h_outs = [res.results[c]["h_out"] for c in range(NCORES)]
    out = host_finish(h_outs, inputs["W_fc"], inputs["b_fc"],
                      inputs["bn_gamma"], inputs["bn_beta"])
    return out, res


def kernel(**inputs):
    out, _ = run_cores(inputs, Tn=T_FULL, trace=False)
    return out


# revision 25
# speedup vs baseline: 1.6483x; 1.6483x over previous
"""GRU-D Trainium2 kernel (8-core SPMD, data-parallel over batch).

Model (reference): B=512, T=200, D=128, H=512.
Per-core: 64 batch samples, full T recurrence.

v3: single fused pass, bf16 matmuls, latency-restructured scan.

Decomposition
-------------
h-independent terms are precomputed per 8-step chunk (phase A), kept in
SBUF (no DRAM round trip):
    delta_x = min(1, exp(-(d*w_gx + b_gx)))                  [elementwise]
    xhat    = m*x + (1-m)*(delta_x*xl + (1-delta_x)*xm)      [elementwise]
    dh      = min(1, exp(-(Wgh @ d + b_gh)))                 [D->H matmul]
    P_g     = Wgx_g @ xhat + Wgm_g @ m + b_g   for g in r,z,h

The serial scan consumes those records; per step (g = dh_t * h):
    r|z = sigmoid(P_{r,z} + W{r,z}h @ g)
    u  = r * g
    ht = tanh(P_h + Whh @ u)
    h' = g + z*(ht - g);   g_next = dh_{t+1} * h'
To shorten the post-tanh chain the scan tracks g (not h):
    a = dh' * g, b = dh' * z   (GpSimd, off critical path)
    g_next = a + b*(ht - g)    (3 half-width DVE ops, pipelined per
                                128-col pair so next step's matmuls
                                start on the first half of g_next)
P records are injected into PSUM with an identity matmul (start=True)
so sigmoid/tanh read PSUM directly (no DVE pre-add).

Phase A matmuls of chunk ci+1 are interleaved between scan steps of
chunk ci: they fill PE stall gaps and keep the HAM clock gate released
(PE stays at 2.4 GHz instead of the cold 1.2 GHz).

Everything on-device is feature-major: [H, B_local] tensors live as
SBUF tiles [128, 4*64] with column index = h_chunk*64 + b.

Final projection (H->2) + batch norm run on host over the gathered
h_last (trivial FLOPs, needs cross-core batch statistics anyway).
"""

import sys

for _p in ("/opt/trn_rl_repo",):
    if _p not in sys.path:
        sys.path.insert(0, _p)

import numpy as np
from ml_dtypes import bfloat16

import concourse.bacc as bacc
import concourse.tile as tile
from concourse import mybir

AF = mybir.ActivationFunctionType
F32 = mybir.dt.float32
BF16 = mybir.dt.bfloat16

B, T_FULL, D, H = 512, 200, 128, 512
NCORES = 8
BL = B // NCORES          # 64 samples per core
MC = H // 128             # 4 h-chunks
W = MC * BL               # 256 state columns
CHUNK = 512               # phase-A columns per chunk (= 8 steps)
TPC = CHUNK // BL         # timesteps per chunk (8)
BN_EPS = 1e-5

_nc_cache = {}


def build(T=T_FULL):
    assert T % TPC == 0
    TB = T * BL
    nchunk = TB // CHUNK

    nc = bacc.Bacc("TRN2", target_bir_lowering=False, debug=False)

    def din(name, shape, dt=F32):
        return nc.dram_tensor(name, shape, dt, kind="ExternalInput")

    x_d = din("x", [128, TB], BF16)
    xl_d = din("xl", [128, TB], BF16)
    m_d = din("m", [128, TB], BF16)
    dt_d = din("dt", [128, TB], BF16)
    xm_d = din("xm", [128, TB], BF16)

    wgx_d = din("wgx_n", [128, 1])      # -w_gx
    bgx_d = din("bgx_n", [128, 1])      # -b_gx
    wgh_d = din("wgh_t", [128, H], BF16)   # Wgh.T
    bgh_d = din("bgh_n", [128, MC])     # -b_gh  (col = h chunk)
    eye_d = din("eye", [128, 128], BF16)

    wxs_d = din("wx_t", [128, 3 * H], BF16)   # [Wrx.T | Wzx.T | Whx.T]
    wms_d = din("wm_t", [128, 3 * H], BF16)   # [Wrm.T | Wzm.T | Whm.T]
    whh_d = din("wh_t", [128, 3 * MC * H], BF16)  # r|z|h hidden blocks,
    #                                   tile (k,m) at g*4096 + k*512 + m*128
    bia_d = din("bias", [128, 3 * MC])  # b_r | b_z | b_h  (col = g*4 + chunk)

    h_out = nc.dram_tensor("h_out", [128, W], F32, kind="ExternalOutput")

    xh_s = nc.dram_tensor("xh_s", [128, TB], BF16)     # xhat scratch
    dh_s = nc.dram_tensor("dh_s", [128, T * W], BF16)  # delta_h scratch

    with tile.TileContext(nc) as tc:
        with (
            tc.tile_pool(name="wsb", bufs=1) as wp,
            tc.tile_pool(name="pin", bufs=3) as pin,
            tc.tile_pool(name="paw", bufs=2) as paw,
            tc.tile_pool(name="prec", bufs=2) as prp,
            tc.tile_pool(name="pg", bufs=2) as pg,
            tc.tile_pool(name="pb", bufs=2) as pb,
            tc.tile_pool(name="psA", bufs=2, space="PSUM") as psA,
            tc.tile_pool(name="psR", bufs=2, space="PSUM") as psR,
            tc.tile_pool(name="psZ", bufs=2, space="PSUM") as psZ,
            tc.tile_pool(name="psH", bufs=2, space="PSUM") as psH,
        ):
            # resident weights
            wgx = wp.tile([128, 1], F32, tag="wgx")
            bgx = wp.tile([128, 1], F32, tag="bgx")
            wgh = wp.tile([128, H], BF16, tag="wgh")
            bgh = wp.tile([128, MC], F32, tag="bgh")
            eye = wp.tile([128, 128], BF16, tag="eye")
            wxs = wp.tile([128, 3 * H], BF16, tag="wxs")
            wms = wp.tile([128, 3 * H], BF16, tag="wms")
            whh = wp.tile([128, 3 * MC * H], BF16, tag="whh")
            bia = wp.tile([128, 3 * MC], F32, tag="bia")
            for sb_t, dr in [
                (wgx, wgx_d), (bgx, bgx_d), (wgh, wgh_d), (bgh, bgh_d),
                (eye, eye_d), (wxs, wxs_d), (wms, wms_d), (whh, whh_d),
                (bia, bia_d),
            ]:
                nc.sync.dma_start(sb_t[:], dr[:])

            # ---- phase A0: xhat + delta_h (all Exp work) -> DRAM -------
            for ci in range(nchunk):
                s = ci * CHUNK
                xt = pin.tile([128, CHUNK], BF16, tag="x")
                xlt = pin.tile([128, CHUNK], BF16, tag="xl")
                mt = pin.tile([128, CHUNK], BF16, tag="m")
                dtt = pin.tile([128, CHUNK], BF16, tag="d")
                xmt = pin.tile([128, CHUNK], BF16, tag="xm")
                nc.sync.dma_start(xt[:], x_d[:, s:s + CHUNK])
                nc.sync.dma_start(xlt[:], xl_d[:, s:s + CHUNK])
                nc.sync.dma_start(mt[:], m_d[:, s:s + CHUNK])
                nc.sync.dma_start(dtt[:], dt_d[:, s:s + CHUNK])
                nc.sync.dma_start(xmt[:], xm_d[:, s:s + CHUNK])

                # delta_h = min(1, exp(-(Wgh@d + b)))  [4 matmuls + drains]
                dh_sb = paw.tile([128, TPC, W], BF16, tag="dh")
                for mi in range(MC):
                    pdm = psA.tile([128, CHUNK], F32, tag="psA")
                    nc.tensor.matmul(
                        pdm[:], wgh[:, mi * 128:(mi + 1) * 128], dtt[:],
                        start=True, stop=True)
                    nc.scalar.activation(
                        dh_sb[:, :, mi * BL:(mi + 1) * BL],
                        pdm[:].rearrange("p (t b) -> p t b", b=BL),
                        AF.Exp, bias=bgh[:, mi:mi + 1], scale=-1.0)
                dh_fl = dh_sb[:].rearrange("p t b -> p (t b)")
                nc.vector.tensor_scalar_min(dh_fl, dh_fl, 1.0)
                nc.sync.dma_start(
                    dh_s[:, ci * TPC * W:(ci + 1) * TPC * W], dh_fl)

                # xhat chain
                dxe = paw.tile([128, CHUNK], F32, tag="dxe")
                nc.scalar.activation(dxe[:], dtt[:], AF.Exp,
                                     bias=bgx[:, 0:1], scale=wgx[:, 0:1])
                dx = paw.tile([128, CHUNK], F32, tag="dx")
                nc.vector.tensor_scalar_min(dx[:], dxe[:], 1.0)
                t1 = paw.tile([128, CHUNK], F32, tag="t1")
                nc.gpsimd.tensor_sub(t1[:], xlt[:], xmt[:])
                t2 = paw.tile([128, CHUNK], F32, tag="t2")
                nc.gpsimd.tensor_mul(t2[:], dx[:], t1[:])
                t2b = paw.tile([128, CHUNK], F32, tag="t2b")
                nc.gpsimd.tensor_add(t2b[:], t2[:], xmt[:])
                t3 = paw.tile([128, CHUNK], F32, tag="t3")
                nc.vector.tensor_sub(t3[:], xt[:], t2b[:])
                t4 = paw.tile([128, CHUNK], F32, tag="t4")
                nc.vector.tensor_mul(t4[:], mt[:], t3[:])
                xh = paw.tile([128, CHUNK], BF16, tag="xh")
                nc.vector.tensor_add(xh[:], t4[:], t2b[:])
                nc.sync.dma_start(xh_s[:, s:s + CHUNK], xh[:])

            # ---- fused-loop chunk helpers ------------------------------
            def chunk_start(ci):
                """DMA xh/m/dh for chunk ci; returns state dict."""
                s = ci * CHUNK
                xht = pin.tile([128, CHUNK], BF16, tag="xhf")
                mt = pin.tile([128, CHUNK], BF16, tag="mf")
                dht = pin.tile([128, TPC * W], BF16, tag="dhf")
                nc.sync.dma_start(xht[:], xh_s[:, s:s + CHUNK])
                nc.sync.dma_start(mt[:], m_d[:, s:s + CHUNK])
                nc.sync.dma_start(
                    dht[:], dh_s[:, ci * TPC * W:(ci + 1) * TPC * W])
                prec = prp.tile([128, TPC, 3 * W], BF16, tag="prec")
                return dict(xht=xht, mt=mt, dht=dht, prec=prec)

            def chunk_mms(st):
                """P_g matmul + bias-drain closures (doled out between
                scan steps)."""
                ops = []

                def pg_mm(gi, mi):
                    def f():
                        pp = psA.tile([128, CHUNK], F32, tag="psA")
                        wcol = gi * H + mi * 128
                        nc.tensor.matmul(
                            pp[:], wms[:, wcol:wcol + 128], st["mt"][:],
                            start=True, stop=False)
                        nc.tensor.matmul(
                            pp[:], wxs[:, wcol:wcol + 128], st["xht"][:],
                            start=False, stop=True)
                        dst = st["prec"][:, :, gi * W + mi * BL:
                                         gi * W + (mi + 1) * BL]
                        src = pp[:].rearrange("p (t b) -> p t b", b=BL)
                        b_ap = bia[:, gi * MC + mi:gi * MC + mi + 1]
                        if (gi * MC + mi) % 2 == 0:
                            nc.scalar.activation(dst, src, AF.Identity,
                                                 bias=b_ap)
                        else:
                            nc.vector.tensor_scalar_add(dst, src, b_ap)
                    return f

                for gi in range(3):
                    for mi in range(MC):
                        ops.append(pg_mm(gi, mi))
                return ops

            # ---- the fused scan loop -----------------------------------
            g_cur = pg.tile([128, W], BF16, tag="g0")
            nc.vector.memset(g_cur[:], 0.0)

            stA = chunk_start(0)
            for f in chunk_mms(stA):
                f()

            GATE_R, GATE_Z, GATE_H = 0, 1, 2

            def step(t, st_cur, st_next, apool):
                """One scan step; st_cur holds records for t, st_next for
                t+1 (same chunk or next). apool: phase-A closures to
                interleave. Returns new g tile."""
                nonlocal g_cur
                s = t % TPC
                prec = st_cur["prec"]
                last = t == T - 1
                if not last:
                    s_n = (t + 1) % TPC
                    dh_next = st_next["dht"][:, s_n * W:(s_n + 1) * W]

                def drain(n):
                    for _ in range(n):
                        if apool:
                            apool.pop(0)()

                # --- r gate ---
                pr = psR.tile([128, W], F32, tag="pr")
                nc.tensor.matmul(pr[:], eye[:], prec[:, s, 0:W],
                                 start=True, stop=False)
                for k in range(MC):
                    gk = g_cur[:, k * BL:(k + 1) * BL]
                    for mi in range(MC):
                        wcol = GATE_R * MC * H + k * H + mi * 128
                        nc.tensor.matmul(
                            pr[:, mi * BL:(mi + 1) * BL],
                            whh[:, wcol:wcol + 128], gk,
                            start=False,
                            stop=(k == MC - 1 and mi == MC - 1))
                r_sb = pb.tile([128, W], F32, tag="r")
                nc.scalar.activation(r_sb[:], pr[:], AF.Sigmoid)
                u = pb.tile([128, W], BF16, tag="u")
                nc.vector.tensor_mul(u[:], r_sb[:], g_cur[:])
                if not last:
                    a_t = pb.tile([128, W], F32, tag="a")
                    nc.gpsimd.tensor_mul(a_t[:], dh_next, g_cur[:])

                # --- z gate (PE busy while sigmoid/u run) ---
                pz = psZ.tile([128, W], F32, tag="pz")
                nc.tensor.matmul(pz[:], eye[:], prec[:, s, W:2 * W],
                                 start=True, stop=False)
                for k in range(MC):
                    gk = g_cur[:, k * BL:(k + 1) * BL]
                    for mi in range(MC):
                        wcol = GATE_Z * MC * H + k * H + mi * 128
                        nc.tensor.matmul(
                            pz[:, mi * BL:(mi + 1) * BL],
                            whh[:, wcol:wcol + 128], gk,
                            start=False,
                            stop=(k == MC - 1 and mi == MC - 1))
                drain(1)
                z_sb = pb.tile([128, W], F32, tag="z")
                nc.scalar.activation(z_sb[:], pz[:], AF.Sigmoid)
                if not last:
                    b_t = pb.tile([128, W], F32, tag="b")
                    nc.gpsimd.tensor_mul(b_t[:], dh_next, z_sb[:])

                # --- candidate ---
                ph = psH.tile([128, W], F32, tag="ph")
                nc.tensor.matmul(ph[:], eye[:], prec[:, s, 2 * W:3 * W],
                                 start=True, stop=False)
                for mi in range(MC):
                    for k in range(MC):
                        wcol = GATE_H * MC * H + k * H + mi * 128
                        nc.tensor.matmul(
                            ph[:, mi * BL:(mi + 1) * BL],
                            whh[:, wcol:wcol + 128],
                            u[:, k * BL:(k + 1) * BL],
                            start=False,
                            stop=(mi == MC - 1 and k == MC - 1))
                drain(1)

                # --- tanh + combine, per 128-col pair ---
                g_new = pg.tile([128, W], BF16, tag=f"g{(t + 1) % 2}")
                hts = pb.tile([128, W], F32, tag="hts")
                d1 = pb.tile([128, W], F32, tag="d1")
                if last:
                    d2 = pb.tile([128, W], F32, tag="d2")
                    hfin = pb.tile([128, W], F32, tag="hfin")
                for half in range(2):
                    c0, c1 = half * 128, (half + 1) * 128
                    nc.scalar.activation(hts[:, c0:c1], ph[:, c0:c1],
                                         AF.Tanh)
                    nc.vector.tensor_sub(d1[:, c0:c1], hts[:, c0:c1],
                                         g_cur[:, c0:c1])
                    if last:
                        nc.vector.tensor_mul(d2[:, c0:c1], z_sb[:, c0:c1],
                                             d1[:, c0:c1])
                        nc.vector.tensor_add(hfin[:, c0:c1], g_cur[:, c0:c1],
                                             d2[:, c0:c1])
                    else:
                        nc.vector.tensor_mul(d1[:, c0:c1], b_t[:, c0:c1],
                                             d1[:, c0:c1])
                        nc.vector.tensor_add(g_new[:, c0:c1], a_t[:, c0:c1],
                                             d1[:, c0:c1])
                if last:
                    nc.sync.dma_start(h_out[:], hfin[:])
                g_cur = g_new

            stB = stA
            apool = []
            for ci in range(nchunk):
                st_next = None
                if ci + 1 < nchunk:
                    st_next = chunk_start(ci + 1)
                    apool = chunk_mms(st_next)
                else:
                    apool = []
                for s in range(TPC):
                    t = ci * TPC + s
                    nxt = st_next if s == TPC - 1 else stB
                    step(t, stB, nxt, apool)
                for f in apool:  # any leftovers
                    f()
                apool = []
                stB = st_next

    nc.compile()
    return nc


def get_nc(T=T_FULL):
    if T not in _nc_cache:
        _nc_cache[T] = build(T)
    return _nc_cache[T]


# ---------------------------------------------------------------- host prep

def _feature_major(a, Tn):
    """[BL, T, D] -> [D, T*BL] with b fastest."""
    return np.ascontiguousarray(
        a.transpose(2, 1, 0), bfloat16).reshape(D, Tn * BL)


def prep_shared(W_gh, b_gh, W_z, b_z, W_r, b_r, W_h, b_h, w_gx, b_gx):
    """Weight arrays shared by all cores (host layout). Gate order r,z,h."""
    def split(Wf):
        return Wf[:, :D], Wf[:, D:D + H], Wf[:, D + H:]

    Wzx, Wzh, Wzm = split(W_z)
    Wrx, Wrh, Wrm = split(W_r)
    Whx, Whh_, Whm = split(W_h)

    def hid_t(Wh):
        # Wh [H, H] -> Wh.T tiles: [128, MC*H] with tile (k,m) at k*H + m*128
        return (
            Wh.T.reshape(MC, 128, H).transpose(1, 0, 2).reshape(128, MC * H)
        )

    f32 = np.float32
    return {
        "wgx_n": np.ascontiguousarray(-w_gx[:, None], f32),
        "bgx_n": np.ascontiguousarray(-b_gx[:, None], f32),
        "wgh_t": np.ascontiguousarray(W_gh.T, bfloat16),
        "bgh_n": np.ascontiguousarray(-b_gh.reshape(MC, 128).T, f32),
        "eye": np.ascontiguousarray(np.eye(128), bfloat16),
        "wx_t": np.ascontiguousarray(
            np.concatenate([Wrx.T, Wzx.T, Whx.T], axis=1), bfloat16),
        "wm_t": np.ascontiguousarray(
            np.concatenate([Wrm.T, Wzm.T, Whm.T], axis=1), bfloat16),
        "wh_t": np.ascontiguousarray(
            np.concatenate([hid_t(Wrh), hid_t(Wzh), hid_t(Whh_)], axis=1),
            bfloat16),
        "bias": np.ascontiguousarray(
            np.concatenate(
                [b.reshape(MC, 128).T for b in (b_r, b_z, b_h)], axis=1), f32),
    }


def prep_core(X, X_last_obsv, Mask, Delta, xm_fm, shared, c, Tn):
    sl = slice(c * BL, (c + 1) * BL)
    m = {
        "x": _feature_major(X[sl], Tn),
        "xl": _feature_major(X_last_obsv[sl], Tn),
        "m": _feature_major(Mask[sl], Tn),
        "dt": _feature_major(Delta[sl], Tn),
        "xm": xm_fm,
    }
    m.update(shared)
    return m


def host_finish(h_outs, W_fc, b_fc, bn_gamma, bn_beta):
    """Gather per-core h_last, project to logits, batch-norm over batch."""
    h_last = np.concatenate(
        [o.reshape(128, MC, BL).transpose(2, 1, 0).reshape(BL, H)
         for o in h_outs], axis=0)                      # [B, H]
    logits = h_last.astype(np.float32) @ W_fc.T.astype(np.float32) + b_fc
    mu = logits.mean(axis=0)
    var = logits.var(axis=0)
    out = bn_gamma * (logits - mu) / np.sqrt(var + BN_EPS) + bn_beta
    return out.astype(np.float32)


def run_cores(inputs, Tn=T_FULL, trace=False):
    from concourse.bass_utils import run_bass_kernel_spmd

    inputs = {k: np.asarray(v, dtype=np.float32) for k, v in inputs.items()}
    nc = get_nc(Tn)
    shared = prep_shared(
        inputs["W_gh"], inputs["b_gh"], inputs["W_z"], inputs["b_z"],
        inputs["W_r"], inputs["b_r"], inputs["W_h"], inputs["b_h"],
        inputs["w_gx"], inputs["b_gx"],
    )
    xm_fm = np.ascontiguousarray(
        np.broadcast_to(
            inputs["x_mean"].transpose(2, 1, 0), (D, Tn, BL)
        ), bfloat16).reshape(D, Tn * BL)
    in_maps = [
        prep_core(inputs["X"], inputs["X_last_obsv"], inputs["Mask"],
                  inputs["Delta"], xm_fm, shared, c, Tn)
        for c in range(NCORES)
    ]
    res = run_bass_kernel_spmd(
        nc, in_maps, list(range(NCORES)), trace=trace,
    )
    h_outs = [res.results[c]["h_out"] for c in range(NCORES)]
    out = host_finish(h_outs, inputs["W_fc"], inputs["b_fc"],
                      inputs["bn_gamma"], inputs["bn_beta"])
    return out, res


def kernel(**inputs):
    out, _ = run_cores(inputs, Tn=T_FULL, trace=False)
    return out


# revision 26
# speedup vs baseline: 1.6539x; 1.0035x over previous
"""GRU-D Trainium2 kernel (8-core SPMD, data-parallel over batch).

Model (reference): B=512, T=200, D=128, H=512.
Per-core: 64 batch samples, full T recurrence.

v3: single fused pass, bf16 matmuls, latency-restructured scan.

Decomposition
-------------
h-independent terms are precomputed per 8-step chunk (phase A), kept in
SBUF (no DRAM round trip):
    delta_x = min(1, exp(-(d*w_gx + b_gx)))                  [elementwise]
    xhat    = m*x + (1-m)*(delta_x*xl + (1-delta_x)*xm)      [elementwise]
    dh      = min(1, exp(-(Wgh @ d + b_gh)))                 [D->H matmul]
    P_g     = Wgx_g @ xhat + Wgm_g @ m + b_g   for g in r,z,h

The serial scan consumes those records; per step (g = dh_t * h):
    r|z = sigmoid(P_{r,z} + W{r,z}h @ g)
    u  = r * g
    ht = tanh(P_h + Whh @ u)
    h' = g + z*(ht - g);   g_next = dh_{t+1} * h'
To shorten the post-tanh chain the scan tracks g (not h):
    a = dh' * g, b = dh' * z   (GpSimd, off critical path)
    g_next = a + b*(ht - g)    (3 half-width DVE ops, pipelined per
                                128-col pair so next step's matmuls
                                start on the first half of g_next)
P records are injected into PSUM with an identity matmul (start=True)
so sigmoid/tanh read PSUM directly (no DVE pre-add).

Phase A matmuls of chunk ci+1 are interleaved between scan steps of
chunk ci: they fill PE stall gaps and keep the HAM clock gate released
(PE stays at 2.4 GHz instead of the cold 1.2 GHz).

Everything on-device is feature-major: [H, B_local] tensors live as
SBUF tiles [128, 4*64] with column index = h_chunk*64 + b.

Final projection (H->2) + batch norm run on host over the gathered
h_last (trivial FLOPs, needs cross-core batch statistics anyway).
"""

import sys

for _p in ("/opt/trn_rl_repo",):
    if _p not in sys.path:
        sys.path.insert(0, _p)

import numpy as np
import ml_dtypes
from ml_dtypes import bfloat16

import concourse.bacc as bacc
import concourse.tile as tile
from concourse import mybir

AF = mybir.ActivationFunctionType
F32 = mybir.dt.float32
BF16 = mybir.dt.bfloat16
FP8 = mybir.dt.float8e3
FP8_SCALE = 256.0

B, T_FULL, D, H = 512, 200, 128, 512
NCORES = 8
BL = B // NCORES          # 64 samples per core
MC = H // 128             # 4 h-chunks
W = MC * BL               # 256 state columns
CHUNK = 512               # phase-A columns per chunk (= 8 steps)
TPC = CHUNK // BL         # timesteps per chunk (8)
BN_EPS = 1e-5

_nc_cache = {}


def build(T=T_FULL):
    assert T % TPC == 0
    TB = T * BL
    nchunk = TB // CHUNK

    nc = bacc.Bacc("TRN2", target_bir_lowering=False, debug=False)

    def din(name, shape, dt=F32):
        return nc.dram_tensor(name, shape, dt, kind="ExternalInput")

    x_d = din("x", [128, TB], BF16)
    xl_d = din("xl", [128, TB], BF16)
    m_d = din("m", [128, TB], BF16)
    dt_d = din("dt", [128, TB], BF16)
    xm_d = din("xm", [128, TB], BF16)

    wgx_d = din("wgx_n", [128, 1])      # -w_gx
    bgx_d = din("bgx_n", [128, 1])      # -b_gx
    wgh_d = din("wgh_t", [128, H], BF16)   # Wgh.T
    bgh_d = din("bgh_n", [128, MC])     # -b_gh  (col = h chunk)
    eye_d = din("eye", [128, 128], BF16)

    wxs_d = din("wx_t", [128, 3 * H], BF16)   # [Wrx.T | Wzx.T | Whx.T]
    wms_d = din("wm_t", [128, 3 * H], BF16)   # [Wrm.T | Wzm.T | Whm.T]
    whh_d = din("wh_t", [128, 3 * MC * H], FP8)   # r|z|h hidden blocks,
    #                                   tile (k,m) at g*4096 + k*512 + m*128
    bia_d = din("bias", [128, 3 * MC])  # b_r | b_z | b_h  (col = g*4 + chunk)

    h_out = nc.dram_tensor("h_out", [128, W], F32, kind="ExternalOutput")

    xh_s = nc.dram_tensor("xh_s", [128, TB], BF16)     # xhat scratch
    dh_s = nc.dram_tensor("dh_s", [128, T * W], BF16)  # delta_h scratch

    with tile.TileContext(nc) as tc:
        with (
            tc.tile_pool(name="wsb", bufs=1) as wp,
            tc.tile_pool(name="pin", bufs=3) as pin,
            tc.tile_pool(name="paw", bufs=2) as paw,
            tc.tile_pool(name="prec", bufs=2) as prp,
            tc.tile_pool(name="pg", bufs=2) as pg,
            tc.tile_pool(name="pb", bufs=2) as pb,
            tc.tile_pool(name="psA", bufs=2, space="PSUM") as psA,
            tc.tile_pool(name="psR", bufs=2, space="PSUM") as psR,
            tc.tile_pool(name="psZ", bufs=2, space="PSUM") as psZ,
            tc.tile_pool(name="psH", bufs=2, space="PSUM") as psH,
        ):
            # resident weights
            wgx = wp.tile([128, 1], F32, tag="wgx")
            bgx = wp.tile([128, 1], F32, tag="bgx")
            wgh = wp.tile([128, H], BF16, tag="wgh")
            bgh = wp.tile([128, MC], F32, tag="bgh")
            eye = wp.tile([128, 128], BF16, tag="eye")
            wxs = wp.tile([128, 3 * H], BF16, tag="wxs")
            wms = wp.tile([128, 3 * H], BF16, tag="wms")
            whh = wp.tile([128, 3 * MC * H], FP8, tag="whh")
            bia = wp.tile([128, 3 * MC], F32, tag="bia")
            for sb_t, dr in [
                (wgx, wgx_d), (bgx, bgx_d), (wgh, wgh_d), (bgh, bgh_d),
                (eye, eye_d), (wxs, wxs_d), (wms, wms_d), (whh, whh_d),
                (bia, bia_d),
            ]:
                nc.sync.dma_start(sb_t[:], dr[:])

            # ---- phase A0: xhat + delta_h (all Exp work) -> DRAM -------
            for ci in range(nchunk):
                s = ci * CHUNK
                xt = pin.tile([128, CHUNK], BF16, tag="x")
                xlt = pin.tile([128, CHUNK], BF16, tag="xl")
                mt = pin.tile([128, CHUNK], BF16, tag="m")
                dtt = pin.tile([128, CHUNK], BF16, tag="d")
                xmt = pin.tile([128, CHUNK], BF16, tag="xm")
                nc.sync.dma_start(xt[:], x_d[:, s:s + CHUNK])
                nc.sync.dma_start(xlt[:], xl_d[:, s:s + CHUNK])
                nc.sync.dma_start(mt[:], m_d[:, s:s + CHUNK])
                nc.sync.dma_start(dtt[:], dt_d[:, s:s + CHUNK])
                nc.sync.dma_start(xmt[:], xm_d[:, s:s + CHUNK])

                # delta_h = min(1, exp(-(Wgh@d + b)))  [4 matmuls + drains]
                dh_sb = paw.tile([128, TPC, W], BF16, tag="dh")
                for mi in range(MC):
                    pdm = psA.tile([128, CHUNK], F32, tag="psA")
                    nc.tensor.matmul(
                        pdm[:], wgh[:, mi * 128:(mi + 1) * 128], dtt[:],
                        start=True, stop=True)
                    nc.scalar.activation(
                        dh_sb[:, :, mi * BL:(mi + 1) * BL],
                        pdm[:].rearrange("p (t b) -> p t b", b=BL),
                        AF.Exp, bias=bgh[:, mi:mi + 1], scale=-1.0)
                dh_fl = dh_sb[:].rearrange("p t b -> p (t b)")
                nc.vector.tensor_scalar_min(dh_fl, dh_fl, 1.0)
                nc.sync.dma_start(
                    dh_s[:, ci * TPC * W:(ci + 1) * TPC * W], dh_fl)

                # xhat chain
                dxe = paw.tile([128, CHUNK], F32, tag="dxe")
                nc.scalar.activation(dxe[:], dtt[:], AF.Exp,
                                     bias=bgx[:, 0:1], scale=wgx[:, 0:1])
                dx = paw.tile([128, CHUNK], F32, tag="dx")
                nc.vector.tensor_scalar_min(dx[:], dxe[:], 1.0)
                t1 = paw.tile([128, CHUNK], F32, tag="t1")
                nc.gpsimd.tensor_sub(t1[:], xlt[:], xmt[:])
                t2 = paw.tile([128, CHUNK], F32, tag="t2")
                nc.gpsimd.tensor_mul(t2[:], dx[:], t1[:])
                t2b = paw.tile([128, CHUNK], F32, tag="t2b")
                nc.gpsimd.tensor_add(t2b[:], t2[:], xmt[:])
                t3 = paw.tile([128, CHUNK], F32, tag="t3")
                nc.vector.tensor_sub(t3[:], xt[:], t2b[:])
                t4 = paw.tile([128, CHUNK], F32, tag="t4")
                nc.vector.tensor_mul(t4[:], mt[:], t3[:])
                xh = paw.tile([128, CHUNK], BF16, tag="xh")
                nc.vector.tensor_add(xh[:], t4[:], t2b[:])
                nc.sync.dma_start(xh_s[:, s:s + CHUNK], xh[:])

            # ---- fused-loop chunk helpers ------------------------------
            def chunk_start(ci):
                """DMA xh/m/dh for chunk ci; returns state dict."""
                s = ci * CHUNK
                xht = pin.tile([128, CHUNK], BF16, tag="xhf")
                mt = pin.tile([128, CHUNK], BF16, tag="mf")
                dht = pin.tile([128, TPC * W], BF16, tag="dhf")
                nc.sync.dma_start(xht[:], xh_s[:, s:s + CHUNK])
                nc.sync.dma_start(mt[:], m_d[:, s:s + CHUNK])
                nc.sync.dma_start(
                    dht[:], dh_s[:, ci * TPC * W:(ci + 1) * TPC * W])
                prec = prp.tile([128, TPC, 3 * W], BF16, tag="prec")
                return dict(xht=xht, mt=mt, dht=dht, prec=prec)

            def chunk_mms(st):
                """P_g matmul + bias-drain closures (doled out between
                scan steps)."""
                ops = []

                def pg_mm(gi, mi):
                    def f():
                        pp = psA.tile([128, CHUNK], F32, tag="psA")
                        wcol = gi * H + mi * 128
                        nc.tensor.matmul(
                            pp[:], wms[:, wcol:wcol + 128], st["mt"][:],
                            start=True, stop=False)
                        nc.tensor.matmul(
                            pp[:], wxs[:, wcol:wcol + 128], st["xht"][:],
                            start=False, stop=True)
                        dst = st["prec"][:, :, gi * W + mi * BL:
                                         gi * W + (mi + 1) * BL]
                        src = pp[:].rearrange("p (t b) -> p t b", b=BL)
                        b_ap = bia[:, gi * MC + mi:gi * MC + mi + 1]
                        if (gi * MC + mi) % 2 == 0:
                            nc.scalar.activation(dst, src, AF.Identity,
                                                 bias=b_ap)
                        else:
                            nc.vector.tensor_scalar_add(dst, src, b_ap)
                    return f

                for gi in range(3):
                    for mi in range(MC):
                        ops.append(pg_mm(gi, mi))
                return ops

            # ---- the fused scan loop -----------------------------------
            g_cur = pg.tile([128, W], BF16, tag="g0")
            nc.vector.memset(g_cur[:], 0.0)

            stA = chunk_start(0)
            for f in chunk_mms(stA):
                f()

            GATE_R, GATE_Z, GATE_H = 0, 1, 2

            def step(t, st_cur, st_next, apool):
                """One scan step; st_cur holds records for t, st_next for
                t+1 (same chunk or next). apool: phase-A closures to
                interleave. Returns new g tile."""
                nonlocal g_cur
                s = t % TPC
                prec = st_cur["prec"]
                last = t == T - 1
                if not last:
                    s_n = (t + 1) % TPC
                    dh_next = st_next["dht"][:, s_n * W:(s_n + 1) * W]

                def drain(n):
                    for _ in range(n):
                        if apool:
                            apool.pop(0)()

                # --- r gate ---
                pr = psR.tile([128, W], F32, tag="pr")
                nc.tensor.matmul(pr[:], eye[:], prec[:, s, 0:W],
                                 start=True, stop=False)
                for k in range(MC):
                    gk = g_cur[:, k * BL:(k + 1) * BL]
                    for mi in range(MC):
                        wcol = GATE_R * MC * H + k * H + mi * 128
                        nc.tensor.matmul(
                            pr[:, mi * BL:(mi + 1) * BL],
                            whh[:, wcol:wcol + 128], gk,
                            start=False,
                            stop=(k == MC - 1 and mi == MC - 1))
                r_sb = pb.tile([128, W], F32, tag="r")
                nc.scalar.activation(r_sb[:], pr[:], AF.Sigmoid, scale=1.0 / FP8_SCALE)
                u = pb.tile([128, W], BF16, tag="u")
                nc.vector.tensor_mul(u[:], r_sb[:], g_cur[:])
                if not last:
                    a_t = pb.tile([128, W], F32, tag="a")
                    nc.gpsimd.tensor_mul(a_t[:], dh_next, g_cur[:])

                # --- z gate (PE busy while sigmoid/u run) ---
                pz = psZ.tile([128, W], F32, tag="pz")
                nc.tensor.matmul(pz[:], eye[:], prec[:, s, W:2 * W],
                                 start=True, stop=False)
                for k in range(MC):
                    gk = g_cur[:, k * BL:(k + 1) * BL]
                    for mi in range(MC):
                        wcol = GATE_Z * MC * H + k * H + mi * 128
                        nc.tensor.matmul(
                            pz[:, mi * BL:(mi + 1) * BL],
                            whh[:, wcol:wcol + 128], gk,
                            start=False,
                            stop=(k == MC - 1 and mi == MC - 1))
                drain(1)
                z_sb = pb.tile([128, W], F32, tag="z")
                nc.scalar.activation(z_sb[:], pz[:], AF.Sigmoid, scale=1.0 / FP8_SCALE)
                if not last:
                    b_t = pb.tile([128, W], F32, tag="b")
                    nc.gpsimd.tensor_mul(b_t[:], dh_next, z_sb[:])

                # --- candidate ---
                ph = psH.tile([128, W], F32, tag="ph")
                nc.tensor.matmul(ph[:], eye[:], prec[:, s, 2 * W:3 * W],
                                 start=True, stop=False)
                for mi in range(MC):
                    for k in range(MC):
                        wcol = GATE_H * MC * H + k * H + mi * 128
                        nc.tensor.matmul(
                            ph[:, mi * BL:(mi + 1) * BL],
                            whh[:, wcol:wcol + 128],
                            u[:, k * BL:(k + 1) * BL],
                            start=False,
                            stop=(mi == MC - 1 and k == MC - 1))
                drain(1)

                # --- tanh + combine, per 128-col pair ---
                g_new = pg.tile([128, W], BF16, tag=f"g{(t + 1) % 2}")
                hts = pb.tile([128, W], F32, tag="hts")
                d1 = pb.tile([128, W], F32, tag="d1")
                if last:
                    d2 = pb.tile([128, W], F32, tag="d2")
                    hfin = pb.tile([128, W], F32, tag="hfin")
                for half in range(2):
                    c0, c1 = half * 128, (half + 1) * 128
                    nc.scalar.activation(hts[:, c0:c1], ph[:, c0:c1],
                                         AF.Tanh, scale=1.0 / FP8_SCALE)
                    nc.vector.tensor_sub(d1[:, c0:c1], hts[:, c0:c1],
                                         g_cur[:, c0:c1])
                    if last:
                        nc.vector.tensor_mul(d2[:, c0:c1], z_sb[:, c0:c1],
                                             d1[:, c0:c1])
                        nc.vector.tensor_add(hfin[:, c0:c1], g_cur[:, c0:c1],
                                             d2[:, c0:c1])
                    else:
                        nc.vector.tensor_mul(d1[:, c0:c1], b_t[:, c0:c1],
                                             d1[:, c0:c1])
                        nc.vector.tensor_add(g_new[:, c0:c1], a_t[:, c0:c1],
                                             d1[:, c0:c1])
                if last:
                    nc.sync.dma_start(h_out[:], hfin[:])
                g_cur = g_new

            stB = stA
            apool = []
            for ci in range(nchunk):
                st_next = None
                if ci + 1 < nchunk:
                    st_next = chunk_start(ci + 1)
                    apool = chunk_mms(st_next)
                else:
                    apool = []
                for s in range(TPC):
                    t = ci * TPC + s
                    nxt = st_next if s == TPC - 1 else stB
                    step(t, stB, nxt, apool)
                for f in apool:  # any leftovers
                    f()
                apool = []
                stB = st_next

    nc.compile()
    return nc


def get_nc(T=T_FULL):
    if T not in _nc_cache:
        _nc_cache[T] = build(T)
    return _nc_cache[T]


# ---------------------------------------------------------------- host prep

def _feature_major(a, Tn):
    """[BL, T, D] -> [D, T*BL] with b fastest."""
    return np.ascontiguousarray(
        a.transpose(2, 1, 0), bfloat16).reshape(D, Tn * BL)


def prep_shared(W_gh, b_gh, W_z, b_z, W_r, b_r, W_h, b_h, w_gx, b_gx):
    """Weight arrays shared by all cores (host layout). Gate order r,z,h."""
    def split(Wf):
        return Wf[:, :D], Wf[:, D:D + H], Wf[:, D + H:]

    Wzx, Wzh, Wzm = split(W_z)
    Wrx, Wrh, Wrm = split(W_r)
    Whx, Whh_, Whm = split(W_h)

    def hid_t(Wh):
        # Wh [H, H] -> Wh.T tiles: [128, MC*H] with tile (k,m) at k*H + m*128
        return (
            Wh.T.reshape(MC, 128, H).transpose(1, 0, 2).reshape(128, MC * H)
        )

    f32 = np.float32
    return {
        "wgx_n": np.ascontiguousarray(-w_gx[:, None], f32),
        "bgx_n": np.ascontiguousarray(-b_gx[:, None], f32),
        "wgh_t": np.ascontiguousarray(W_gh.T, bfloat16),
        "bgh_n": np.ascontiguousarray(-b_gh.reshape(MC, 128).T, f32),
        "eye": np.ascontiguousarray(np.eye(128), bfloat16),
        "wx_t": np.ascontiguousarray(
            FP8_SCALE * np.concatenate([Wrx.T, Wzx.T, Whx.T], axis=1),
            bfloat16),
        "wm_t": np.ascontiguousarray(
            FP8_SCALE * np.concatenate([Wrm.T, Wzm.T, Whm.T], axis=1),
            bfloat16),
        "wh_t": np.ascontiguousarray(
            FP8_SCALE * np.concatenate(
                [hid_t(Wrh), hid_t(Wzh), hid_t(Whh_)], axis=1),
            ml_dtypes.float8_e3m4),
        "bias": np.ascontiguousarray(
            FP8_SCALE * np.concatenate(
                [b.reshape(MC, 128).T for b in (b_r, b_z, b_h)], axis=1), f32),
    }


def prep_core(X, X_last_obsv, Mask, Delta, xm_fm, shared, c, Tn):
    sl = slice(c * BL, (c + 1) * BL)
    m = {
        "x": _feature_major(X[sl], Tn),
        "xl": _feature_major(X_last_obsv[sl], Tn),
        "m": _feature_major(Mask[sl], Tn),
        "dt": _feature_major(Delta[sl], Tn),
        "xm": xm_fm,
    }
    m.update(shared)
    return m


def host_finish(h_outs, W_fc, b_fc, bn_gamma, bn_beta):
    """Gather per-core h_last, project to logits, batch-norm over batch."""
    h_last = np.concatenate(
        [o.reshape(128, MC, BL).transpose(2, 1, 0).reshape(BL, H)
         for o in h_outs], axis=0)                      # [B, H]
    logits = h_last.astype(np.float32) @ W_fc.T.astype(np.float32) + b_fc
    mu = logits.mean(axis=0)
    var = logits.var(axis=0)
    out = bn_gamma * (logits - mu) / np.sqrt(var + BN_EPS) + bn_beta
    return out.astype(np.float32)


def run_cores(inputs, Tn=T_FULL, trace=False):
    from concourse.bass_utils import run_bass_kernel_spmd

    inputs = {k: np.asarray(v, dtype=np.float32) for k, v in inputs.items()}
    nc = get_nc(Tn)
    shared = prep_shared(
        inputs["W_gh"], inputs["b_gh"], inputs["W_z"], inputs["b_z"],
        inputs["W_r"], inputs["b_r"], inputs["W_h"], inputs["b_h"],
        inputs["w_gx"], inputs["b_gx"],
    )
    xm_fm = np.ascontiguousarray(
        np.broadcast_to(
            inputs["x_mean"].transpose(2, 1, 0), (D, Tn, BL)
        ), bfloat16).reshape(D, Tn * BL)
    in_maps = [
        prep_core(inputs["X"], inputs["X_last_obsv"], inputs["Mask"],
                  inputs["Delta"], xm_fm, shared, c, Tn)
        for c in range(NCORES)
    ]
    res = run_bass_kernel_spmd(
        nc, in_maps, list(range(NCORES)), trace=trace,
    )
    h_outs = [res.results[c]["h_out"] for c in range(NCORES)]
    out = host_finish(h_outs, inputs["W_fc"], inputs["b_fc"],
                      inputs["bn_gamma"], inputs["bn_beta"])
    return out, res


def kernel(**inputs):
    out, _ = run_cores(inputs, Tn=T_FULL, trace=False)
    return out


# revision 29
# speedup vs baseline: 1.6584x; 1.0027x over previous
"""GRU-D Trainium2 kernel (8-core SPMD, data-parallel over batch).

Model (reference): B=512, T=200, D=128, H=512.
Per-core: 64 batch samples, full T recurrence.

v3: single fused pass, bf16 matmuls, latency-restructured scan.

Decomposition
-------------
h-independent terms are precomputed per 8-step chunk (phase A), kept in
SBUF (no DRAM round trip):
    delta_x = min(1, exp(-(d*w_gx + b_gx)))                  [elementwise]
    xhat    = m*x + (1-m)*(delta_x*xl + (1-delta_x)*xm)      [elementwise]
    dh      = min(1, exp(-(Wgh @ d + b_gh)))                 [D->H matmul]
    P_g     = Wgx_g @ xhat + Wgm_g @ m + b_g   for g in r,z,h

The serial scan consumes those records; per step (g = dh_t * h):
    r|z = sigmoid(P_{r,z} + W{r,z}h @ g)
    u  = r * g
    ht = tanh(P_h + Whh @ u)
    h' = g + z*(ht - g);   g_next = dh_{t+1} * h'
To shorten the post-tanh chain the scan tracks g (not h):
    a = dh' * g, b = dh' * z   (GpSimd, off critical path)
    g_next = a + b*(ht - g)    (3 half-width DVE ops, pipelined per
                                128-col pair so next step's matmuls
                                start on the first half of g_next)
P records are injected into PSUM with an identity matmul (start=True)
so sigmoid/tanh read PSUM directly (no DVE pre-add).

Phase A matmuls of chunk ci+1 are interleaved between scan steps of
chunk ci: they fill PE stall gaps and keep the HAM clock gate released
(PE stays at 2.4 GHz instead of the cold 1.2 GHz).

Everything on-device is feature-major: [H, B_local] tensors live as
SBUF tiles [128, 4*64] with column index = h_chunk*64 + b.

Final projection (H->2) + batch norm run on host over the gathered
h_last (trivial FLOPs, needs cross-core batch statistics anyway).
"""

import sys

for _p in ("/opt/trn_rl_repo",):
    if _p not in sys.path:
        sys.path.insert(0, _p)

import numpy as np
import ml_dtypes
from ml_dtypes import bfloat16

import concourse.bacc as bacc
import concourse.tile as tile
from concourse import mybir

AF = mybir.ActivationFunctionType
F32 = mybir.dt.float32
BF16 = mybir.dt.bfloat16
FP8 = mybir.dt.float8e3
FP8_SCALE = 256.0

B, T_FULL, D, H = 512, 200, 128, 512
NCORES = 8
BL = B // NCORES          # 64 samples per core
MC = H // 128             # 4 h-chunks
W = MC * BL               # 256 state columns
CHUNK = 512               # phase-A columns per chunk (= 8 steps)
TPC = CHUNK // BL         # timesteps per chunk (8)
BN_EPS = 1e-5

_nc_cache = {}


def build(T=T_FULL):
    assert T % TPC == 0
    TB = T * BL
    nchunk = TB // CHUNK

    nc = bacc.Bacc("TRN2", target_bir_lowering=False, debug=False)

    def din(name, shape, dt=F32):
        return nc.dram_tensor(name, shape, dt, kind="ExternalInput")

    x_d = din("x", [128, TB], BF16)
    xl_d = din("xl", [128, TB], BF16)
    m_d = din("m", [128, TB], BF16)
    dt_d = din("dt", [128, TB], BF16)
    xm_d = din("xm", [128, TB], BF16)

    wgx_d = din("wgx_n", [128, 1])      # -w_gx
    bgx_d = din("bgx_n", [128, 1])      # -b_gx
    wgh_d = din("wgh_t", [128, H], BF16)   # Wgh.T
    bgh_d = din("bgh_n", [128, MC])     # -b_gh  (col = h chunk)
    eye_d = din("eye", [128, 128], BF16)

    wxs_d = din("wx_t", [128, 3 * H], BF16)   # [Wrx.T | Wzx.T | Whx.T]
    wms_d = din("wm_t", [128, 3 * H], BF16)   # [Wrm.T | Wzm.T | Whm.T]
    whh_d = din("wh_t", [128, 3 * MC * H], FP8)   # r|z|h hidden blocks,
    #                                   tile (k,m) at g*4096 + k*512 + m*128
    bia_d = din("bias", [128, 3 * MC])  # b_r | b_z | b_h  (col = g*4 + chunk)

    h_out = nc.dram_tensor("h_out", [128, W], F32, kind="ExternalOutput")

    xh_s = nc.dram_tensor("xh_s", [128, TB], BF16)     # xhat scratch
    dh_s = nc.dram_tensor("dh_s", [128, T * W], BF16)  # delta_h scratch

    with tile.TileContext(nc) as tc:
        with (
            tc.tile_pool(name="wsb", bufs=1) as wp,
            tc.tile_pool(name="pin", bufs=3) as pin,
            tc.tile_pool(name="paw", bufs=2) as paw,
            tc.tile_pool(name="prec", bufs=2) as prp,
            tc.tile_pool(name="pg", bufs=2) as pg,
            tc.tile_pool(name="pb", bufs=2) as pb,
            tc.tile_pool(name="psA", bufs=2, space="PSUM") as psA,
            tc.tile_pool(name="psR", bufs=2, space="PSUM") as psR,
            tc.tile_pool(name="psZ", bufs=2, space="PSUM") as psZ,
            tc.tile_pool(name="psH", bufs=2, space="PSUM") as psH,
        ):
            # resident weights
            wgx = wp.tile([128, 1], F32, tag="wgx")
            bgx = wp.tile([128, 1], F32, tag="bgx")
            wgh = wp.tile([128, H], BF16, tag="wgh")
            bgh = wp.tile([128, MC], F32, tag="bgh")
            eye = wp.tile([128, 128], BF16, tag="eye")
            wxs = wp.tile([128, 3 * H], BF16, tag="wxs")
            wms = wp.tile([128, 3 * H], BF16, tag="wms")
            whh = wp.tile([128, 3 * MC * H], FP8, tag="whh")
            bia = wp.tile([128, 3 * MC], F32, tag="bia")
            for sb_t, dr in [
                (wgx, wgx_d), (bgx, bgx_d), (wgh, wgh_d), (bgh, bgh_d),
                (eye, eye_d), (wxs, wxs_d), (wms, wms_d), (whh, whh_d),
                (bia, bia_d),
            ]:
                nc.sync.dma_start(sb_t[:], dr[:])

            # ---- phase A0: xhat + delta_h (all Exp work) -> DRAM -------
            for ci in range(nchunk):
                s = ci * CHUNK
                xt = pin.tile([128, CHUNK], BF16, tag="x")
                xlt = pin.tile([128, CHUNK], BF16, tag="xl")
                mt = pin.tile([128, CHUNK], BF16, tag="m")
                dtt = pin.tile([128, CHUNK], BF16, tag="d")
                xmt = pin.tile([128, CHUNK], BF16, tag="xm")
                nc.sync.dma_start(xt[:], x_d[:, s:s + CHUNK])
                nc.sync.dma_start(xlt[:], xl_d[:, s:s + CHUNK])
                nc.sync.dma_start(mt[:], m_d[:, s:s + CHUNK])
                nc.sync.dma_start(dtt[:], dt_d[:, s:s + CHUNK])
                nc.sync.dma_start(xmt[:], xm_d[:, s:s + CHUNK])

                # delta_h = min(1, exp(-(Wgh@d + b)))  [4 matmuls + drains]
                dh_sb = paw.tile([128, TPC, W], BF16, tag="dh")
                for mi in range(MC):
                    pdm = psA.tile([128, CHUNK], F32, tag="psA")
                    nc.tensor.matmul(
                        pdm[:], wgh[:, mi * 128:(mi + 1) * 128], dtt[:],
                        start=True, stop=True)
                    nc.scalar.activation(
                        dh_sb[:, :, mi * BL:(mi + 1) * BL],
                        pdm[:].rearrange("p (t b) -> p t b", b=BL),
                        AF.Exp, bias=bgh[:, mi:mi + 1], scale=-1.0)
                dh_fl = dh_sb[:].rearrange("p t b -> p (t b)")
                nc.vector.tensor_scalar_min(dh_fl, dh_fl, 1.0)
                nc.sync.dma_start(
                    dh_s[:, ci * TPC * W:(ci + 1) * TPC * W], dh_fl)

                # xhat chain
                dxe = paw.tile([128, CHUNK], F32, tag="dxe")
                nc.scalar.activation(dxe[:], dtt[:], AF.Exp,
                                     bias=bgx[:, 0:1], scale=wgx[:, 0:1])
                dx = paw.tile([128, CHUNK], F32, tag="dx")
                nc.vector.tensor_scalar_min(dx[:], dxe[:], 1.0)
                t1 = paw.tile([128, CHUNK], F32, tag="t1")
                nc.gpsimd.tensor_sub(t1[:], xlt[:], xmt[:])
                t2 = paw.tile([128, CHUNK], F32, tag="t2")
                nc.gpsimd.tensor_mul(t2[:], dx[:], t1[:])
                t2b = paw.tile([128, CHUNK], F32, tag="t2b")
                nc.gpsimd.tensor_add(t2b[:], t2[:], xmt[:])
                t3 = paw.tile([128, CHUNK], F32, tag="t3")
                nc.vector.tensor_sub(t3[:], xt[:], t2b[:])
                t4 = paw.tile([128, CHUNK], F32, tag="t4")
                nc.vector.tensor_mul(t4[:], mt[:], t3[:])
                xh = paw.tile([128, CHUNK], BF16, tag="xh")
                nc.vector.tensor_add(xh[:], t4[:], t2b[:])
                nc.sync.dma_start(xh_s[:, s:s + CHUNK], xh[:])

            # ---- fused-loop chunk helpers ------------------------------
            def chunk_start(ci):
                """DMA xh/m/dh for chunk ci; returns state dict."""
                s = ci * CHUNK
                xht = pin.tile([128, CHUNK], BF16, tag="xhf")
                mt = pin.tile([128, CHUNK], BF16, tag="mf")
                dht = pin.tile([128, TPC * W], BF16, tag="dhf")
                nc.sync.dma_start(xht[:], xh_s[:, s:s + CHUNK])
                nc.sync.dma_start(mt[:], m_d[:, s:s + CHUNK])
                nc.sync.dma_start(
                    dht[:], dh_s[:, ci * TPC * W:(ci + 1) * TPC * W])
                prec = prp.tile([128, 3 * MC, CHUNK], BF16, tag="prec")
                return dict(xht=xht, mt=mt, dht=dht, prec=prec)

            def chunk_mms(st):
                """P_g matmul + bias-drain closures (doled out between
                scan steps)."""
                ops = []

                def pg_mm(gi, mi):
                    def f():
                        pp = psA.tile([128, CHUNK], F32, tag="psA")
                        wcol = gi * H + mi * 128
                        nc.tensor.matmul(
                            pp[:], wms[:, wcol:wcol + 128], st["mt"][:],
                            start=True, stop=False)
                        nc.tensor.matmul(
                            pp[:], wxs[:, wcol:wcol + 128], st["xht"][:],
                            start=False, stop=True)
                        dst = st["prec"][:, gi * MC + mi, :]
                        b_ap = bia[:, gi * MC + mi:gi * MC + mi + 1]
                        if (gi * MC + mi) % 2 == 0:
                            nc.scalar.activation(dst, pp[:], AF.Identity,
                                                 bias=b_ap)
                        else:
                            nc.vector.tensor_scalar_add(dst, pp[:], b_ap)
                    return f

                for gi in range(3):
                    for mi in range(MC):
                        ops.append(pg_mm(gi, mi))
                return ops

            # ---- the fused scan loop -----------------------------------
            g_cur = pg.tile([128, W], BF16, tag="g0")
            nc.vector.memset(g_cur[:], 0.0)

            stA = chunk_start(0)
            for f in chunk_mms(stA):
                f()

            GATE_R, GATE_Z, GATE_H = 0, 1, 2

            def step(t, st_cur, st_next, apool):
                """One scan step; st_cur holds records for t, st_next for
                t+1 (same chunk or next). apool: phase-A closures to
                interleave. Returns new g tile."""
                nonlocal g_cur
                s = t % TPC
                prec = st_cur["prec"]
                last = t == T - 1
                if not last:
                    s_n = (t + 1) % TPC
                    dh_next = st_next["dht"][:, s_n * W:(s_n + 1) * W]

                def drain(n):
                    for _ in range(n):
                        if apool:
                            apool.pop(0)()

                # --- r gate ---
                pr = psR.tile([128, W], F32, tag="pr")
                nc.tensor.matmul(
                    pr[:], eye[:],
                    prec[:, 0:MC, s * BL:(s + 1) * BL],
                    start=True, stop=False)
                for k in range(MC):
                    gk = g_cur[:, k * BL:(k + 1) * BL]
                    for mi in range(MC):
                        wcol = GATE_R * MC * H + k * H + mi * 128
                        nc.tensor.matmul(
                            pr[:, mi * BL:(mi + 1) * BL],
                            whh[:, wcol:wcol + 128], gk,
                            start=False,
                            stop=(k == MC - 1 and mi == MC - 1))
                r_sb = pb.tile([128, W], F32, tag="r")
                nc.scalar.activation(r_sb[:], pr[:], AF.Sigmoid, scale=1.0 / FP8_SCALE)
                u = pb.tile([128, W], BF16, tag="u")
                nc.vector.tensor_mul(u[:], r_sb[:], g_cur[:])
                if not last:
                    a_t = pb.tile([128, W], F32, tag="a")
                    nc.gpsimd.tensor_mul(a_t[:], dh_next, g_cur[:])

                # --- z gate (PE busy while sigmoid/u run) ---
                pz = psZ.tile([128, W], F32, tag="pz")
                nc.tensor.matmul(
                    pz[:], eye[:],
                    prec[:, MC:2 * MC, s * BL:(s + 1) * BL],
                    start=True, stop=False)
                for k in range(MC):
                    gk = g_cur[:, k * BL:(k + 1) * BL]
                    for mi in range(MC):
                        wcol = GATE_Z * MC * H + k * H + mi * 128
                        nc.tensor.matmul(
                            pz[:, mi * BL:(mi + 1) * BL],
                            whh[:, wcol:wcol + 128], gk,
                            start=False,
                            stop=(k == MC - 1 and mi == MC - 1))
                drain(1)
                z_sb = pb.tile([128, W], F32, tag="z")
                nc.scalar.activation(z_sb[:], pz[:], AF.Sigmoid, scale=1.0 / FP8_SCALE)
                if not last:
                    b_t = pb.tile([128, W], F32, tag="b")
                    nc.gpsimd.tensor_mul(b_t[:], dh_next, z_sb[:])
                    # c = a - b*g = dh'*(1-z)*g  (off critical path)
                    bg = pb.tile([128, W], F32, tag="bg")
                    nc.gpsimd.tensor_mul(bg[:], b_t[:], g_cur[:])
                    c_t = pb.tile([128, W], F32, tag="c")
                    nc.gpsimd.tensor_sub(c_t[:], a_t[:], bg[:])

                # --- candidate ---
                ph = psH.tile([128, W], F32, tag="ph")
                nc.tensor.matmul(
                    ph[:], eye[:],
                    prec[:, 2 * MC:3 * MC, s * BL:(s + 1) * BL],
                    start=True, stop=False)
                for mi in range(MC):
                    for k in range(MC):
                        wcol = GATE_H * MC * H + k * H + mi * 128
                        nc.tensor.matmul(
                            ph[:, mi * BL:(mi + 1) * BL],
                            whh[:, wcol:wcol + 128],
                            u[:, k * BL:(k + 1) * BL],
                            start=False,
                            stop=(mi == MC - 1 and k == MC - 1))
                drain(1)

                # --- tanh + combine, per 128-col pair ---
                g_new = pg.tile([128, W], BF16, tag=f"g{(t + 1) % 2}")
                hts = pb.tile([128, W], F32, tag="hts")
                if last:
                    d1 = pb.tile([128, W], F32, tag="d1")
                    d2 = pb.tile([128, W], F32, tag="d2")
                    hfin = pb.tile([128, W], F32, tag="hfin")
                for half in range(2):
                    c0, c1 = half * 128, (half + 1) * 128
                    nc.scalar.activation(hts[:, c0:c1], ph[:, c0:c1],
                                         AF.Tanh, scale=1.0 / FP8_SCALE)
                    if last:
                        nc.vector.tensor_sub(d1[:, c0:c1], hts[:, c0:c1],
                                             g_cur[:, c0:c1])
                        nc.vector.tensor_mul(d2[:, c0:c1], z_sb[:, c0:c1],
                                             d1[:, c0:c1])
                        nc.vector.tensor_add(hfin[:, c0:c1], g_cur[:, c0:c1],
                                             d2[:, c0:c1])
                    else:
                        # g' = c + b*tanh  (2-op tail)
                        nc.vector.tensor_mul(hts[:, c0:c1], b_t[:, c0:c1],
                                             hts[:, c0:c1])
                        nc.vector.tensor_add(g_new[:, c0:c1], c_t[:, c0:c1],
                                             hts[:, c0:c1])
                if last:
                    nc.sync.dma_start(h_out[:], hfin[:])
                g_cur = g_new

            stB = stA
            apool = []
            for ci in range(nchunk):
                st_next = None
                if ci + 1 < nchunk:
                    st_next = chunk_start(ci + 1)
                    apool = chunk_mms(st_next)
                else:
                    apool = []
                for s in range(TPC):
                    t = ci * TPC + s
                    nxt = st_next if s == TPC - 1 else stB
                    step(t, stB, nxt, apool)
                for f in apool:  # any leftovers
                    f()
                apool = []
                stB = st_next

    nc.compile()
    return nc


def get_nc(T=T_FULL):
    if T not in _nc_cache:
        _nc_cache[T] = build(T)
    return _nc_cache[T]


# ---------------------------------------------------------------- host prep

def _feature_major(a, Tn):
    """[BL, T, D] -> [D, T*BL] with b fastest."""
    return np.ascontiguousarray(
        a.transpose(2, 1, 0), bfloat16).reshape(D, Tn * BL)


def prep_shared(W_gh, b_gh, W_z, b_z, W_r, b_r, W_h, b_h, w_gx, b_gx):
    """Weight arrays shared by all cores (host layout). Gate order r,z,h."""
    def split(Wf):
        return Wf[:, :D], Wf[:, D:D + H], Wf[:, D + H:]

    Wzx, Wzh, Wzm = split(W_z)
    Wrx, Wrh, Wrm = split(W_r)
    Whx, Whh_, Whm = split(W_h)

    def hid_t(Wh):
        # Wh [H, H] -> Wh.T tiles: [128, MC*H] with tile (k,m) at k*H + m*128
        return (
            Wh.T.reshape(MC, 128, H).transpose(1, 0, 2).reshape(128, MC * H)
        )

    f32 = np.float32
    return {
        "wgx_n": np.ascontiguousarray(-w_gx[:, None], f32),
        "bgx_n": np.ascontiguousarray(-b_gx[:, None], f32),
        "wgh_t": np.ascontiguousarray(W_gh.T, bfloat16),
        "bgh_n": np.ascontiguousarray(-b_gh.reshape(MC, 128).T, f32),
        "eye": np.ascontiguousarray(np.eye(128), bfloat16),
        "wx_t": np.ascontiguousarray(
            FP8_SCALE * np.concatenate([Wrx.T, Wzx.T, Whx.T], axis=1),
            bfloat16),
        "wm_t": np.ascontiguousarray(
            FP8_SCALE * np.concatenate([Wrm.T, Wzm.T, Whm.T], axis=1),
            bfloat16),
        "wh_t": np.ascontiguousarray(
            FP8_SCALE * np.concatenate(
                [hid_t(Wrh), hid_t(Wzh), hid_t(Whh_)], axis=1),
            ml_dtypes.float8_e3m4),
        "bias": np.ascontiguousarray(
            FP8_SCALE * np.concatenate(
                [b.reshape(MC, 128).T for b in (b_r, b_z, b_h)], axis=1), f32),
    }


def prep_core(X, X_last_obsv, Mask, Delta, xm_fm, shared, c, Tn):
    sl = slice(c * BL, (c + 1) * BL)
    m = {
        "x": _feature_major(X[sl], Tn),
        "xl": _feature_major(X_last_obsv[sl], Tn),
        "m": _feature_major(Mask[sl], Tn),
        "dt": _feature_major(Delta[sl], Tn),
        "xm": xm_fm,
    }
    m.update(shared)
    return m


def host_finish(h_outs, W_fc, b_fc, bn_gamma, bn_beta):
    """Gather per-core h_last, project to logits, batch-norm over batch."""
    h_last = np.concatenate(
        [o.reshape(128, MC, BL).transpose(2, 1, 0).reshape(BL, H)
         for o in h_outs], axis=0)                      # [B, H]
    logits = h_last.astype(np.float32) @ W_fc.T.astype(np.float32) + b_fc
    mu = logits.mean(axis=0)
    var = logits.var(axis=0)
    out = bn_gamma * (logits - mu) / np.sqrt(var + BN_EPS) + bn_beta
    return out.astype(np.float32)


def run_cores(inputs, Tn=T_FULL, trace=False):
    from concourse.bass_utils import run_bass_kernel_spmd

    inputs = {k: np.asarray(v, dtype=np.float32) for k, v in inputs.items()}
    nc = get_nc(Tn)
    shared = prep_shared(
        inputs["W_gh"], inputs["b_gh"], inputs["W_z"], inputs["b_z"],
        inputs["W_r"], inputs["b_r"], inputs["W_h"], inputs["b_h"],
        inputs["w_gx"], inputs["b_gx"],
    )
    xm_fm = np.ascontiguousarray(
        np.broadcast_to(
            inputs["x_mean"].transpose(2, 1, 0), (D, Tn, BL)
        ), bfloat16).reshape(D, Tn * BL)
    in_maps = [
        prep_core(inputs["X"], inputs["X_last_obsv"], inputs["Mask"],
                  inputs["Delta"], xm_fm, shared, c, Tn)
        for c in range(NCORES)
    ]
    res = run_bass_kernel_spmd(
        nc, in_maps, list(range(NCORES)), trace=trace,
    )
    h_outs = [res.results[c]["h_out"] for c in range(NCORES)]
    out = host_finish(h_outs, inputs["W_fc"], inputs["b_fc"],
                      inputs["bn_gamma"], inputs["bn_beta"])
    return out, res


def kernel(**inputs):
    out, _ = run_cores(inputs, Tn=T_FULL, trace=False)
    return out
